# revision 3
# baseline (speedup 1.0000x reference)
"""ButterflyLinear Trainium2 kernel.

Math insight: every one of the 12 butterfly stages pairs features strictly
within aligned groups of 4 (stage 0 pairs (4k,4k+1),(4k+2,4k+3); stages 1..11
all pair (4k,4k+2),(4k+1,4k+3)).  The whole network therefore collapses
exactly to a block-diagonal linear map with 1024 independent 4x4 blocks:

    out[t, 4k+j] = sum_i x[t, 4k+i] * M_k[i, j] + bias[4k+j]

M is extracted on the host (float64) by pushing the 4 group-basis vectors
through the stage chain, then the device kernel is a single pass over x:
per 128-token x 128-feature tile, PE-transpose x, then a 128x128 matmul with
the (block-diagonal) weight chunk, bias added during the PSUM->SBUF copy.

Sharding: data-parallel over tokens, 8192/8 = 1024 tokens per core.
"""

import numpy as np

TOKENS = 8192
N = 4096
DEPTH = 12
NCORES = 8
TOK_PER_CORE = TOKENS // NCORES  # 1024
P = 128          # partitions / tile edge
N_CHUNKS = N // P       # 32 feature chunks of 128
GROUP = 4        # feature-chunks packed per PSUM bank (4*128 = 512 fp32)
N_GROUPS = N_CHUNKS // GROUP  # 8
N_TTILES = TOK_PER_CORE // P  # 8 token tiles per core


def _apply_stage_np(x, factor, stage):
    B, n = x.shape
    block = 1 << (stage + 1)
    half = block >> 1
    m = n // block
    staged = x.reshape(B, m, half, 2).transpose(0, 1, 3, 2)
    pairs = staged.reshape(B, n // 2, 2)
    t = np.einsum("bnc,ncd->bnd", pairs, factor)
    t = t.reshape(B, m, 2, half).transpose(0, 1, 3, 2)
    return t.reshape(B, n)


def _compose_weights(factors):
    """Return M_cols [4, N] float64: M_cols[i, m] = Mfull[4*(m//4)+i, m]."""
    V = np.zeros((4, N), dtype=np.float64)
    for i in range(4):
        V[i, i::4] = 1.0
    M = V
    f64 = np.asarray(factors, dtype=np.float64)
    for s in range(DEPTH):
        M = _apply_stage_np(M, f64[s], s)
    return M


def _build_wmat(factors):
    """Dense [128, N] fp32 weight: wmat[p, c*128+q] = Mfull[c*128+p, c*128+q].

    Column block c of 128 is the (block-diagonal) 128x128 weight for feature
    chunk c; nonzero only where p//4 == q//4 within the chunk.
    """
    M_cols = _compose_weights(factors)  # [4, N]
    wmat = np.zeros((P, N), dtype=np.float64)
    p = np.arange(P)
    q = np.arange(P)
    same_block = (p[:, None] // 4) == (q[None, :] // 4)  # [128,128]
    for c in range(N_CHUNKS):
        cols = M_cols[:, c * P:(c + 1) * P]       # [4, 128]
        block = cols[p % 4, :]                    # [128, 128] = M_cols[p%4, q]
        wmat[:, c * P:(c + 1) * P] = np.where(same_block, block, 0.0)
    return np.ascontiguousarray(wmat.astype(np.float32))


_PROG = None


def _get_program():
    global _PROG
    if _PROG is not None:
        return _PROG

    import concourse.bass as bass
    import concourse.mybir as mybir
    import concourse.tile as tile
    from concourse import bacc
    from concourse.masks import make_identity

    nc = bacc.Bacc("TRN2", target_bir_lowering=False, debug=False,
                   num_devices=NCORES)
    f32 = mybir.dt.float32
    xs_h = nc.dram_tensor("xs", [TOK_PER_CORE, N], f32, kind="ExternalInput")
    w_h = nc.dram_tensor("wmat", [P, N], f32, kind="ExternalInput")
    b_h = nc.dram_tensor("bias", [N], f32, kind="ExternalInput")
    out_h = nc.dram_tensor("out", [TOK_PER_CORE, N], f32, kind="ExternalOutput")

    xs = xs_h.ap()
    out = out_h.ap()
    bias = b_h.ap()

    with tile.TileContext(nc) as tc:
        with (
            tc.tile_pool(name="singles", bufs=1) as singles,
            tc.tile_pool(name="xin", bufs=2) as xpool,
            tc.tile_pool(name="oout", bufs=2) as opool,
            tc.tile_pool(name="xt", bufs=3) as xtpool,
            tc.tile_pool(name="psA", bufs=2, space="PSUM") as psa,
            tc.tile_pool(name="psB", bufs=2, space="PSUM") as psb,
        ):
            ident = singles.tile([P, P], f32)
            make_identity(nc, ident)
            w_sb = singles.tile([P, N], f32)
            nc.sync.dma_start(out=w_sb, in_=w_h.ap())
            bias_sb = singles.tile([P, N], f32)
            bias_bcast = bass.AP(
                tensor=bias.tensor, offset=bias.offset,
                ap=[[0, P], [1, N]],
            )
            nc.gpsimd.dma_start(out=bias_sb, in_=bias_bcast)

            for t in range(N_TTILES):
                x_t = xpool.tile([P, N], f32)
                nc.sync.dma_start(out=x_t, in_=xs[t * P:(t + 1) * P, :])
                out_t = opool.tile([P, N], f32)
                for g in range(N_GROUPS):
                    xt_ps = psa.tile([P, GROUP * P], f32)
                    for cl in range(GROUP):
                        c = g * GROUP + cl
                        nc.tensor.transpose(
                            xt_ps[:, cl * P:(cl + 1) * P],
                            x_t[:, c * P:(c + 1) * P],
                            ident,
                        )
                    xt_sb = xtpool.tile([P, GROUP * P], f32)
                    nc.scalar.copy(xt_sb, xt_ps)
                    out_ps = psb.tile([P, GROUP * P], f32)
                    for cl in range(GROUP):
                        c = g * GROUP + cl
                        nc.tensor.matmul(
                            out_ps[:, cl * P:(cl + 1) * P],
                            lhsT=xt_sb[:, cl * P:(cl + 1) * P],
                            rhs=w_sb[:, c * P:(c + 1) * P],
                            start=True, stop=True,
                        )
                    nc.vector.tensor_add(
                        out_t[:, g * GROUP * P:(g + 1) * GROUP * P],
                        out_ps,
                        bias_sb[:, g * GROUP * P:(g + 1) * GROUP * P],
                    )
                nc.sync.dma_start(out=out[t * P:(t + 1) * P, :], in_=out_t)

    nc.compile()
    _PROG = nc
    return nc


def kernel(x, factors, bias):
    from concourse.bass_utils import run_bass_kernel_spmd

    x = np.ascontiguousarray(np.asarray(x, dtype=np.float32))
    factors = np.asarray(factors, dtype=np.float32)
    bias_np = np.ascontiguousarray(np.asarray(bias, dtype=np.float32))
    assert x.shape == (TOKENS, N)

    wmat = _build_wmat(factors)

    nc = _get_program()
    in_maps = []
    for c in range(NCORES):
        in_maps.append({
            "xs": np.ascontiguousarray(
                x[c * TOK_PER_CORE:(c + 1) * TOK_PER_CORE]),
            "wmat": wmat,
            "bias": bias_np,
        })
    res = run_bass_kernel_spmd(nc, in_maps, core_ids=list(range(NCORES)))
    return np.concatenate(
        [res.results[c]["out"] for c in range(NCORES)], axis=0)


# revision 5
# speedup vs baseline: 1.1803x; 1.1803x over previous
"""ButterflyLinear Trainium2 kernel.

Math insight: every one of the 12 butterfly stages pairs features strictly
within aligned groups of 4 (stage 0 pairs (4k,4k+1),(4k+2,4k+3); stages 1..11
all pair (4k,4k+2),(4k+1,4k+3)).  The whole network therefore collapses
exactly to a block-diagonal linear map with 1024 independent 4x4 blocks:

    out[t, 4k+j] = sum_i x[t, 4k+i] * M_k[i, j] + bias[4k+j]

M is extracted on the host (float64) by pushing the 4 group-basis vectors
through the stage chain.  The device kernel is a feature-major matmul pass:
the host ships x pre-transposed (feature-major tiles, 16KB-contiguous rows),
each 128-feature chunk is one stationary-weight matmul
out_c[of, tok] = W_c.T @ x_c[if, tok] with N=512 tokens moving, bias added
per-partition during the PSUM->SBUF copy, and the host un-transposes the
returned output.  No on-device transposes, no identity, no bias broadcast.

Sharding: data-parallel over tokens, 8192/8 = 1024 tokens per core.
"""

import numpy as np

TOKENS = 8192
N = 4096
DEPTH = 12
NCORES = 8
TOK_PER_CORE = TOKENS // NCORES  # 1024
P = 128                  # partitions
N_CHUNKS = N // P        # 32 feature chunks of 128
GROUP = 4                # chunks per x/out group tile (4*1024 tok = 16KB rows)
N_GROUPS = N_CHUNKS // GROUP   # 8
TBLK = 512               # moving-token block per matmul (fp32 N<=512)
N_TBLK = TOK_PER_CORE // TBLK  # 2


def _apply_stage_np(x, factor, stage):
    B, n = x.shape
    block = 1 << (stage + 1)
    half = block >> 1
    m = n // block
    staged = x.reshape(B, m, half, 2).transpose(0, 1, 3, 2)
    pairs = staged.reshape(B, n // 2, 2)
    t = np.einsum("bnc,ncd->bnd", pairs, factor)
    t = t.reshape(B, m, 2, half).transpose(0, 1, 3, 2)
    return t.reshape(B, n)


def _compose_weights(factors):
    """Return M_cols [4, N] float64: M_cols[i, m] = Mfull[4*(m//4)+i, m]."""
    V = np.zeros((4, N), dtype=np.float64)
    for i in range(4):
        V[i, i::4] = 1.0
    M = V
    f64 = np.asarray(factors, dtype=np.float64)
    for s in range(DEPTH):
        M = _apply_stage_np(M, f64[s], s)
    return M


def _build_wmat(factors):
    """Dense [128, N] fp32 weight: wmat[p, c*128+q] = Mfull[c*128+p, c*128+q].

    Column block c is the (block-diagonal) 128x128 stationary weight for
    feature chunk c (lhsT layout [if, of]); nonzero only where p//4 == q//4.
    """
    M_cols = _compose_weights(factors)  # [4, N]
    wmat = np.zeros((P, N), dtype=np.float64)
    p = np.arange(P)
    q = np.arange(P)
    same_block = (p[:, None] // 4) == (q[None, :] // 4)
    for c in range(N_CHUNKS):
        cols = M_cols[:, c * P:(c + 1) * P]       # [4, 128]
        block = cols[p % 4, :]                    # block[p, q] = M_cols[p%4, q]
        wmat[:, c * P:(c + 1) * P] = np.where(same_block, block, 0.0)
    return np.ascontiguousarray(wmat.astype(np.float32))


_PROG = None


def _get_program():
    global _PROG
    if _PROG is not None:
        return _PROG

    import concourse.mybir as mybir
    import concourse.tile as tile
    from concourse import bacc

    nc = bacc.Bacc("TRN2", target_bir_lowering=False, debug=False,
                   num_devices=NCORES)
    f32 = mybir.dt.float32
    xp_h = nc.dram_tensor("xp", [N_GROUPS, P, GROUP * TOK_PER_CORE], f32,
                          kind="ExternalInput")
    w_h = nc.dram_tensor("wmat", [P, N], f32, kind="ExternalInput")
    bt_h = nc.dram_tensor("biast", [P, N_CHUNKS], f32, kind="ExternalInput")
    op_h = nc.dram_tensor("outp", [N_GROUPS, P, GROUP * TOK_PER_CORE], f32,
                          kind="ExternalOutput")

    xp = xp_h.ap()
    op = op_h.ap()
    w = w_h.ap()

    with tile.TileContext(nc) as tc:
        with (
            tc.tile_pool(name="singles", bufs=1) as singles,
            tc.tile_pool(name="wts", bufs=2) as wpool,
            tc.tile_pool(name="xin", bufs=2) as xpool,
            tc.tile_pool(name="oout", bufs=2) as opool,
            tc.tile_pool(name="ps", bufs=4, space="PSUM") as pspool,
        ):
            bias_sb = singles.tile([P, N_CHUNKS], f32)
            nc.gpsimd.dma_start(out=bias_sb, in_=bt_h.ap())

            for g in range(N_GROUPS):
                wg = wpool.tile([P, GROUP * P], f32)
                nc.sync.dma_start(
                    out=wg, in_=w[:, g * GROUP * P:(g + 1) * GROUP * P])
                xg = xpool.tile([P, GROUP * TOK_PER_CORE], f32)
                nc.sync.dma_start(out=xg, in_=xp[g])
                og = opool.tile([P, GROUP * TOK_PER_CORE], f32)
                for cc in range(GROUP):
                    c = g * GROUP + cc
                    for tb in range(N_TBLK):
                        ps = pspool.tile([P, TBLK], f32)
                        nc.tensor.matmul(
                            ps,
                            lhsT=wg[:, cc * P:(cc + 1) * P],
                            rhs=xg[:, cc * TOK_PER_CORE + tb * TBLK:
                                   cc * TOK_PER_CORE + (tb + 1) * TBLK],
                            start=True, stop=True,
                        )
                        dst = og[:, cc * TOK_PER_CORE + tb * TBLK:
                                 cc * TOK_PER_CORE + (tb + 1) * TBLK]
                        bcol = bias_sb[:, c:c + 1]
                        if (cc + tb) % 2 == 0:
                            nc.scalar.activation(
                                dst, ps,
                                mybir.ActivationFunctionType.Identity,
                                bias=bcol,
                            )
                        else:
                            nc.vector.tensor_scalar_add(dst, ps, bcol)
                nc.sync.dma_start(out=op[g], in_=og)

    nc.compile()
    _PROG = nc
    return nc


def _prep_core_input(xs):
    """[1024, 4096] token-major -> [8, 128, 4096] feature-major group tiles.

    xprep[g, p, cc*1024 + t] = xs[t, (4g+cc)*128 + p]
    """
    xt = xs.T.reshape(N_GROUPS, GROUP, P, TOK_PER_CORE)   # [g][cc][p][t]
    return np.ascontiguousarray(
        xt.transpose(0, 2, 1, 3).reshape(N_GROUPS, P, GROUP * TOK_PER_CORE))


def _unprep_core_output(outp):
    """Inverse of _prep_core_input for the output tensor."""
    o = outp.reshape(N_GROUPS, P, GROUP, TOK_PER_CORE).transpose(0, 2, 1, 3)
    return o.reshape(N, TOK_PER_CORE).T   # [1024, 4096] token-major view


def kernel(x, factors, bias):
    from concourse.bass_utils import run_bass_kernel_spmd

    x = np.asarray(x, dtype=np.float32)
    factors = np.asarray(factors, dtype=np.float32)
    bias_np = np.asarray(bias, dtype=np.float32)
    assert x.shape == (TOKENS, N)

    wmat = _build_wmat(factors)
    biast = np.ascontiguousarray(bias_np.reshape(N_CHUNKS, P).T)

    nc = _get_program()
    in_maps = []
    for c in range(NCORES):
        in_maps.append({
            "xp": _prep_core_input(x[c * TOK_PER_CORE:(c + 1) * TOK_PER_CORE]),
            "wmat": wmat,
            "biast": biast,
        })
    res = run_bass_kernel_spmd(nc, in_maps, core_ids=list(range(NCORES)))
    out = np.empty((TOKENS, N), dtype=np.float32)
    for c in range(NCORES):
        out[c * TOK_PER_CORE:(c + 1) * TOK_PER_CORE] = _unprep_core_output(
            res.results[c]["outp"])
    return out


# revision 6
# speedup vs baseline: 1.3996x; 1.1858x over previous
"""ButterflyLinear Trainium2 kernel.

Math insight: every one of the 12 butterfly stages pairs features strictly
within aligned groups of 4 (stage 0 pairs (4k,4k+1),(4k+2,4k+3); stages 1..11
all pair (4k,4k+2),(4k+1,4k+3)).  The whole network therefore collapses
exactly to a block-diagonal linear map with 1024 independent 4x4 blocks:

    out[t, 4k+j] = sum_i x[t, 4k+i] * M_k[i, j] + bias[4k+j]

M is extracted on the host (float64) by pushing the 4 group-basis vectors
through the stage chain.  The device kernel is a feature-major matmul pass:
the host ships x pre-transposed (feature-major tiles, 16KB-contiguous rows),
each 128-feature chunk is one stationary-weight matmul
out_c[of, tok] = W_c.T @ x_c[if, tok] with N=512 tokens moving, bias added
per-partition during the PSUM->SBUF copy, and the host un-transposes the
returned output.  No on-device transposes, no identity, no bias broadcast.

Sharding: data-parallel over tokens, 8192/8 = 1024 tokens per core.
"""

import numpy as np

TOKENS = 8192
N = 4096
DEPTH = 12
NCORES = 8
TOK_PER_CORE = TOKENS // NCORES  # 1024
P = 128                  # partitions
N_CHUNKS = N // P        # 32 feature chunks of 128
GROUP = 4                # chunks per x/out group tile (4*1024 tok = 16KB rows)
N_GROUPS = N_CHUNKS // GROUP   # 8
TBLK = 512               # moving-token block per matmul (fp32 N<=512)
N_TBLK = TOK_PER_CORE // TBLK  # 2


def _apply_stage_np(x, factor, stage):
    B, n = x.shape
    block = 1 << (stage + 1)
    half = block >> 1
    m = n // block
    staged = x.reshape(B, m, half, 2).transpose(0, 1, 3, 2)
    pairs = staged.reshape(B, n // 2, 2)
    t = np.einsum("bnc,ncd->bnd", pairs, factor)
    t = t.reshape(B, m, 2, half).transpose(0, 1, 3, 2)
    return t.reshape(B, n)


def _compose_weights(factors):
    """Return M_cols [4, N] float64: M_cols[i, m] = Mfull[4*(m//4)+i, m]."""
    V = np.zeros((4, N), dtype=np.float64)
    for i in range(4):
        V[i, i::4] = 1.0
    M = V
    f64 = np.asarray(factors, dtype=np.float64)
    for s in range(DEPTH):
        M = _apply_stage_np(M, f64[s], s)
    return M


def _build_wmat(factors):
    """Dense [128, N] fp32 weight: wmat[p, c*128+q] = Mfull[c*128+p, c*128+q].

    Column block c is the (block-diagonal) 128x128 stationary weight for
    feature chunk c (lhsT layout [if, of]); nonzero only where p//4 == q//4.
    """
    M_cols = _compose_weights(factors)  # [4, N]
    wmat = np.zeros((P, N), dtype=np.float64)
    p = np.arange(P)
    q = np.arange(P)
    same_block = (p[:, None] // 4) == (q[None, :] // 4)
    for c in range(N_CHUNKS):
        cols = M_cols[:, c * P:(c + 1) * P]       # [4, 128]
        block = cols[p % 4, :]                    # block[p, q] = M_cols[p%4, q]
        wmat[:, c * P:(c + 1) * P] = np.where(same_block, block, 0.0)
    return np.ascontiguousarray(wmat.astype(np.float32))


_PROG = None


def _get_program():
    global _PROG
    if _PROG is not None:
        return _PROG

    import concourse.mybir as mybir
    import concourse.tile as tile
    from concourse import bacc

    nc = bacc.Bacc("TRN2", target_bir_lowering=False, debug=False,
                   num_devices=NCORES)
    f32 = mybir.dt.float32
    xp_h = nc.dram_tensor("xp", [N_GROUPS, P, GROUP * TOK_PER_CORE], f32,
                          kind="ExternalInput")
    w_h = nc.dram_tensor("wmat", [P, N], f32, kind="ExternalInput")
    bt_h = nc.dram_tensor("biast", [P, N_CHUNKS], f32, kind="ExternalInput")
    op_h = nc.dram_tensor("outp", [N_GROUPS, P, GROUP * TOK_PER_CORE], f32,
                          kind="ExternalOutput")

    xp = xp_h.ap()
    op = op_h.ap()
    w = w_h.ap()

    HGRP = GROUP // 2          # 2 chunks per half-group unit
    HCOLS = HGRP * TOK_PER_CORE  # 2048 columns per unit

    with tile.TileContext(nc) as tc:
        with (
            tc.tile_pool(name="singles", bufs=1) as singles,
            tc.tile_pool(name="xin", bufs=3) as xpool,
            tc.tile_pool(name="oout", bufs=3) as opool,
            tc.tile_pool(name="ps", bufs=4, space="PSUM") as pspool,
        ):
            bias_sb = singles.tile([P, N_CHUNKS], f32)
            nc.gpsimd.dma_start(out=bias_sb, in_=bt_h.ap())
            # Whole stationary-weight matrix up front on the store queue
            # (ACT HWDGE) so it never contends with the x loads on SP.
            w_sb = singles.tile([P, N], f32)
            nc.scalar.dma_start(out=w_sb, in_=w)

            # 16 half-group units: loads stream on nc.sync, stores pace on
            # nc.scalar, so a store waiting for compute never stalls the
            # next load behind it in the same engine queue.
            for u in range(N_GROUPS * 2):
                g, h = divmod(u, 2)
                xg = xpool.tile([P, HCOLS], f32)
                nc.sync.dma_start(out=xg, in_=xp[g, :, h * HCOLS:(h + 1) * HCOLS])
                og = opool.tile([P, HCOLS], f32)
                for cc in range(HGRP):
                    c = g * GROUP + h * HGRP + cc
                    for tb in range(N_TBLK):
                        ps = pspool.tile([P, TBLK], f32)
                        nc.tensor.matmul(
                            ps,
                            lhsT=w_sb[:, c * P:(c + 1) * P],
                            rhs=xg[:, cc * TOK_PER_CORE + tb * TBLK:
                                   cc * TOK_PER_CORE + (tb + 1) * TBLK],
                            start=True, stop=True,
                        )
                        dst = og[:, cc * TOK_PER_CORE + tb * TBLK:
                                 cc * TOK_PER_CORE + (tb + 1) * TBLK]
                        bcol = bias_sb[:, c:c + 1]
                        if (cc + tb) % 2 == 0:
                            nc.scalar.activation(
                                dst, ps,
                                mybir.ActivationFunctionType.Identity,
                                bias=bcol,
                            )
                        else:
                            nc.vector.tensor_scalar_add(dst, ps, bcol)
                nc.scalar.dma_start(
                    out=op[g, :, h * HCOLS:(h + 1) * HCOLS], in_=og)

    nc.compile()
    _PROG = nc
    return nc


def _prep_core_input(xs):
    """[1024, 4096] token-major -> [8, 128, 4096] feature-major group tiles.

    xprep[g, p, cc*1024 + t] = xs[t, (4g+cc)*128 + p]
    """
    xt = xs.T.reshape(N_GROUPS, GROUP, P, TOK_PER_CORE)   # [g][cc][p][t]
    return np.ascontiguousarray(
        xt.transpose(0, 2, 1, 3).reshape(N_GROUPS, P, GROUP * TOK_PER_CORE))


def _unprep_core_output(outp):
    """Inverse of _prep_core_input for the output tensor."""
    o = outp.reshape(N_GROUPS, P, GROUP, TOK_PER_CORE).transpose(0, 2, 1, 3)
    return o.reshape(N, TOK_PER_CORE).T   # [1024, 4096] token-major view


def kernel(x, factors, bias):
    from concourse.bass_utils import run_bass_kernel_spmd

    x = np.asarray(x, dtype=np.float32)
    factors = np.asarray(factors, dtype=np.float32)
    bias_np = np.asarray(bias, dtype=np.float32)
    assert x.shape == (TOKENS, N)

    wmat = _build_wmat(factors)
    biast = np.ascontiguousarray(bias_np.reshape(N_CHUNKS, P).T)

    nc = _get_program()
    in_maps = []
    for c in range(NCORES):
        in_maps.append({
            "xp": _prep_core_input(x[c * TOK_PER_CORE:(c + 1) * TOK_PER_CORE]),
            "wmat": wmat,
            "biast": biast,
        })
    res = run_bass_kernel_spmd(nc, in_maps, core_ids=list(range(NCORES)))
    out = np.empty((TOKENS, N), dtype=np.float32)
    for c in range(NCORES):
        out[c * TOK_PER_CORE:(c + 1) * TOK_PER_CORE] = _unprep_core_output(
            res.results[c]["outp"])
    return out
